# revision 1
# baseline (speedup 1.0000x reference)
"""GNN message-passing (ACM module) Trainium2 kernel — 8 NeuronCores.

Strategy (per sharding hint): shard nodes (rows) across the 8 cores;
edges partitioned by destination row; weights/LN/attention replicated.

Per core, per graph:
  y = A @ X  computed as a chain of one-hot matmuls: edges sorted by
  (dest-tile, src-chunk); per 128-edge batch, lhsT = S^T [128 edges,
  128 dest-rows] (host-precomputed bf16, vals folded in), rhs = G
  [128 edges, 256] gathered rows of x (dma_gather, int16 chunk-local
  indices), accumulated in PSUM per dest tile.
Then dense stage: out_g = leaky(y_g @ W_g), out_mlp = leaky(x @ W_mlp),
LayerNorm-projection attention epilogue, all on-chip.
"""
import os
import sys
import numpy as np

sys.path.insert(0, "/opt/trn_rl_repo")

import ml_dtypes  # noqa: E402

BF16 = ml_dtypes.bfloat16

# problem constants
N = 100000
D = 256
NCORES = 8
RPC = 12544            # rows per core (8*12544 = 100352 padded)
NPAD = RPC * NCORES
TPC = RPC // 128       # 98 dest tiles (of 128 rows) per core
CHUNK = 25088          # source chunk (int16-indexable)
NCH = 4
GRP = 2                # dest tiles per gather group
NGRP = TPC // GRP      # 49
EPS = 1e-5
T = 3.0


def _host_prep(x, graphs, weights):
    """Build all per-core device inputs. graphs = {name: (rows, cols, vals)}."""
    xpad = np.zeros((NPAD, D), np.float32)
    xpad[:N] = x
    xtab = xpad.astype(BF16)                      # gather table, replicated
    xT = np.ascontiguousarray(xpad.T.astype(BF16))  # [256, NPAD]

    per_core = [dict() for _ in range(NCORES)]
    for c in range(NCORES):
        per_core[c]["xtab"] = xtab
        per_core[c]["xT"] = np.ascontiguousarray(
            xT[:, c * RPC:(c + 1) * RPC])

    schedules = {}
    for gname, (rows, cols, vals) in graphs.items():
        rows = np.asarray(rows).astype(np.int64)
        cols = np.asarray(cols).astype(np.int64)
        vals = np.asarray(vals).astype(np.float32)
        core_of = rows // RPC
        # per-core edge lists sorted by (tile, chunk, col)
        edata = []
        counts = np.zeros((NCORES, TPC, NCH), np.int64)
        for c in range(NCORES):
            m = core_of == c
            r = rows[m] - c * RPC
            co = cols[m]
            v = vals[m]
            t = r >> 7
            ch = co // CHUNK
            order = np.lexsort((co, ch, t))
            r, co, v, t, ch = r[order], co[order], v[order], t[order], ch[order]
            np.add.at(counts[c], (t, ch), 1)
            edata.append((r, co, v, t, ch))
        # shared schedule: batches per (tile, chunk)
        mx = counts.max(axis=0)                       # [TPC, NCH]
        B = -(-mx // 128)                             # ceil
        B = np.maximum(B, 1)
        # slot order: group-major, then chunk, then tile-within-group
        S_tc = B * 128
        nslots = int(S_tc.sum())
        nbatch = int(B.sum())
        # slot base for (t, ch) in stream order
        base = np.zeros((TPC, NCH), np.int64)
        off = 0
        for g in range(NGRP):
            for ch in range(NCH):
                for tt in range(GRP):
                    t = g * GRP + tt
                    base[t, ch] = off
                    off += S_tc[t, ch]
        assert off == nslots

        idx_streams, st_streams = [], []
        for c in range(NCORES):
            r, co, v, t, ch = edata[c]
            cnt = counts[c]
            # slot index per edge: base[t,ch] + rank within (t,ch)
            # edges are sorted by (t, ch), so rank = arange - start of group
            starts = np.zeros((TPC, NCH), np.int64)
            flat = cnt.reshape(-1)
            starts.reshape(-1)[:] = np.concatenate(([0], np.cumsum(flat)[:-1]))
            rank = np.arange(len(r)) - starts[t, ch]
            slot = base[t, ch] + rank
            # gather indices (chunk-local), pad slots -> 0
            idx = np.zeros(nslots, np.int16)
            idx[slot] = (co - ch * CHUNK).astype(np.int16)
            iw = np.zeros((16, nslots // 16), np.int16)
            sl = np.arange(nslots)
            iw[sl % 16, sl // 16] = idx
            idx_streams.append(np.tile(iw, (8, 1)))
            # S^T stream bf16: [128, nbatch*128]; batch b occupies columns
            # [128b, 128b+128); st[e_local, 128b + d] = val of the edge at
            # slot 128b + e_local whose dest row (within tile) is d.
            st = np.zeros((128, nslots), np.float32)
            st[slot & 127, (slot >> 7 << 7) + (r & 127)] = v
            st_streams.append(st.astype(BF16))
        schedules[gname] = dict(B=B, nslots=nslots, nbatch=nbatch, base=base)
        for c in range(NCORES):
            per_core[c][f"idx_{gname}"] = idx_streams[c]
            per_core[c][f"st_{gname}"] = st_streams[c]
    return per_core, schedules


def _build(nc_mod, schedules, wl, wh, wm, wrep, W1, Cc, A):
    """Build the Bass graph. wl/wh/wm: [256,256] bf16 np arrays baked as
    consts? -> passed as inputs instead. wrep [128,768] bf16. W1, Cc: len-3
    float lists. A: [3,3] floats."""
    import concourse.bass as bass
    import concourse.mybir as mybir
    import concourse.tile as tile

    nc = nc_mod
    F32 = mybir.dt.float32
    BF = mybir.dt.bfloat16
    AL = mybir.AluOpType
    AF = mybir.ActivationFunctionType

    xtab = nc.dram_tensor("xtab", [NPAD, D], BF, kind="ExternalInput")
    xT = nc.dram_tensor("xT", [D, RPC], BF, kind="ExternalInput")
    w_in = {}
    for nm in ("wlow", "whigh", "wmlp"):
        w_in[nm] = nc.dram_tensor(nm, [D, D], BF, kind="ExternalInput")
    wrep_in = nc.dram_tensor("wrep", [128, 3 * D], BF, kind="ExternalInput")
    gins = {}
    for g in ("low", "high"):
        sch = schedules[g]
        gins[g] = dict(
            idx=nc.dram_tensor(f"idx_{g}", [128, sch["nslots"] // 16],
                               mybir.dt.int16, kind="ExternalInput"),
            st=nc.dram_tensor(f"st_{g}", [128, sch["nslots"]], BF,
                              kind="ExternalInput"),
        )
    out = nc.dram_tensor("out", [RPC, D], F32, kind="ExternalOutput")

    with tile.TileContext(nc) as tc:
        # DRAM scratch for y (bf16, node-major)
        with tc.tile_pool(name="ydram", bufs=1, space="DRAM") as ydp:
            y_d = {g: ydp.tile([RPC, D], BF, name=f"y_{g}", tag=f"y_{g}") for g in ("low", "high")}

            # ---- segment-sum stage per graph ----
            with (
                tc.tile_pool(name="seg", bufs=2) as seg,
                tc.tile_pool(name="segps", bufs=2, space="PSUM") as sps,
                tc.tile_pool(name="ydr", bufs=3) as ydr,
            ):
                for g in ("low", "high"):
                    sch = schedules[g]
                    B = sch["B"]
                    base = sch["base"]
                    for grp in range(NGRP):
                        tiles = [grp * GRP + tt for tt in range(GRP)]
                        s0 = int(base[tiles[0], 0])
                        s1 = int(base[tiles[-1], NCH - 1] +
                                 B[tiles[-1], NCH - 1] * 128)
                        nsl = s1 - s0
                        # one G/idx/st buffer per group (double-buffered pool)
                        g_t = seg.tile([128, nsl // 128, D], BF, tag="G")
                        idx_t = seg.tile([128, nsl // 16], mybir.dt.int16,
                                         tag="idx")
                        st_t = seg.tile([128, nsl], BF, tag="st")
                        nc.sync.dma_start(idx_t[:],
                                          gins[g]["idx"][:, s0 // 16:s1 // 16])
                        nc.sync.dma_start(st_t[:], gins[g]["st"][:, s0:s1])
                        # gathers: one per chunk, covering this group's slots
                        for ch in range(NCH):
                            c0 = int(base[tiles[0], ch])
                            c1 = int(base[tiles[-1], ch] + B[tiles[-1], ch] * 128)
                            nid = c1 - c0
                            nc.gpsimd.dma_gather(
                                out_ap=g_t[:, (c0 - s0) // 128:(c1 - s0) // 128, :],
                                in_ap=xtab[ch * CHUNK:(ch + 1) * CHUNK, :],
                                idxs_ap=idx_t[:, (c0 - s0) // 16:(c1 - s0) // 16],
                                num_idxs=nid, num_idxs_reg=nid,
                                elem_size=D, single_packet=False,
                            )
                        ps = {t: sps.tile([128, D], F32, name=f"ps_{g}_{t}",
                                          tag=f"ps{t % (2 * GRP)}")
                              for t in tiles}
                        for ch in range(NCH):
                            for t in tiles:
                                b0 = int(base[t, ch])
                                for b in range(int(B[t, ch])):
                                    sb = b0 + b * 128 - s0
                                    nc.tensor.matmul(
                                        ps[t][:],
                                        st_t[:, sb:sb + 128],
                                        g_t[:, sb // 128, :],
                                        start=(ch == 0 and b == 0),
                                        stop=(ch == NCH - 1 and
                                              b == int(B[t, ch]) - 1),
                                    )
                        for t in tiles:
                            yt = ydr.tile([128, D], BF, tag="yt")
                            nc.scalar.copy(yt[:], ps[t][:])
                            nc.sync.dma_start(
                                y_d[g][t * 128:(t + 1) * 128, :], yt[:])

            # ---- dense + epilogue stage ----
            with (
                tc.tile_pool(name="dw", bufs=1) as dw,
                tc.tile_pool(name="dloop", bufs=3) as dl,
                tc.tile_pool(name="dps", bufs=2, space="PSUM") as dps,
            ):
                w_t = {}
                for nm in ("wlow", "whigh", "wmlp"):
                    w_t[nm] = dw.tile([128, 2, D], BF, name=f"w_{nm}", tag=nm)
                    for k in range(2):
                        nc.sync.dma_start(w_t[nm][:, k, :],
                                          w_in[nm][k * 128:(k + 1) * 128, :])
                wrep_t = dw.tile([128, 3 * D], BF)
                nc.sync.dma_start(wrep_t[:], wrep_in[:])
                w1rep = dw.tile([128, 3], F32)
                c3rep = dw.tile([128, 3], F32)
                arep = [dw.tile([128, 3], F32, name=f"arep{j}", tag=f"arep{j}")
                        for j in range(3)]
                for j in range(3):
                    nc.gpsimd.memset(w1rep[:, j:j + 1], float(W1[j]))
                    nc.gpsimd.memset(c3rep[:, j:j + 1], float(Cc[j]))
                    for i in range(3):
                        nc.gpsimd.memset(arep[i][:, j:j + 1], float(A[i][j] / T))

                for t in range(TPC):
                    # lhsT tiles: transposed loads of y (and xT direct)
                    ps3 = {}
                    for bi, (gname, wname) in enumerate(
                            (("low", "wlow"), ("high", "whigh"))):
                        for k in range(2):
                            yT = dl.tile([128, 128], BF, tag=f"yT{bi}{k}")
                            nc.sync.dma_start(
                                yT[:],
                                y_d[gname][t * 128:(t + 1) * 128,
                                           k * 128:(k + 1) * 128],
                                transpose=True)
                            if k == 0:
                                ps3[bi] = dps.tile([128, D], F32, name=f"eps{bi}",
                                                   tag=f"eps{bi}")
                            nc.tensor.matmul(
                                ps3[bi][:], yT[:],
                                w_t[wname][:, k, :],
                                start=(k == 0), stop=(k == 1))
                    ps3[2] = dps.tile([128, D], F32, name="eps2t", tag="eps2")
                    for k in range(2):
                        xTt = dl.tile([128, 128], BF, tag=f"xTt{k}")
                        nc.sync.dma_start(
                            xTt[:],
                            xT[k * 128:(k + 1) * 128, t * 128:(t + 1) * 128])
                        nc.tensor.matmul(
                            ps3[2][:], xTt[:],
                            w_t["wmlp"][:, k, :],
                            start=(k == 0), stop=(k == 1))

                    # epilogue
                    zall = dl.tile([128, 3, D], BF, tag="zall")
                    sr3 = dl.tile([128, 3], F32, tag="sr3")
                    sz3 = dl.tile([128, 3], F32, tag="sz3")
                    rsc = dl.tile([128, D], BF, tag="rsc")
                    z01 = dl.tile([128, D], BF, tag="z01")
                    for bi in range(3):
                        nc.scalar.activation(rsc[:], ps3[bi][:], AF.Relu,
                                             accum_out=sr3[:, bi:bi + 1])
                        nc.scalar.activation(z01[:], ps3[bi][:], AF.Copy,
                                             scale=0.01,
                                             accum_out=sz3[:, bi:bi + 1])
                        nc.vector.tensor_tensor(out=zall[:, bi, :], in0=z01[:],
                                                in1=ps3[bi][:], op=AL.max)
                    # sums of leaky: 0.99*sr + sz ; means /256
                    m3 = dl.tile([128, 3], F32, tag="m3")
                    nc.vector.tensor_scalar(out=m3[:], in0=sr3[:],
                                            scalar1=0.99 / D, scalar2=None,
                                            op0=AL.mult)
                    nc.vector.scalar_tensor_tensor(
                        out=m3[:], in0=sz3[:], scalar=1.0 / D, in1=m3[:],
                        op0=AL.mult, op1=AL.add)
                    # sq sums and proj sums
                    sq = dl.tile([128, 3, D], BF, tag="sq")
                    nc.vector.tensor_tensor(out=sq[:], in0=zall[:],
                                            in1=zall[:], op=AL.mult)
                    ss3 = dl.tile([128, 3], F32, tag="ss3")
                    nc.vector.tensor_reduce(ss3[:], sq[:],
                                            axis=mybir.AxisListType.X,
                                            op=AL.add)
                    pj = dl.tile([128, 3, D], BF, tag="pj")
                    nc.vector.tensor_tensor(
                        out=pj[:], in0=zall[:],
                        in1=wrep_t[:].rearrange("p (a d) -> p a d", a=3),
                        op=AL.mult)
                    p3 = dl.tile([128, 3], F32, tag="p3")
                    nc.vector.tensor_reduce(p3[:], pj[:],
                                            axis=mybir.AxisListType.X,
                                            op=AL.add)
                    # var = ss/256 - m^2 ; rstd = sqrt(1/(var+eps))
                    v3 = dl.tile([128, 3], F32, tag="v3")
                    nc.vector.tensor_tensor(out=v3[:], in0=m3[:], in1=m3[:],
                                            op=AL.mult)
                    nc.vector.scalar_tensor_tensor(
                        out=v3[:], in0=ss3[:], scalar=1.0 / D, in1=v3[:],
                        op0=AL.mult, op1=AL.subtract)
                    nc.vector.tensor_scalar(out=v3[:], in0=v3[:], scalar1=EPS,
                                            scalar2=None, op0=AL.add)
                    r3 = dl.tile([128, 3], F32, tag="r3")
                    nc.vector.reciprocal(r3[:], v3[:])
                    rstd3 = dl.tile([128, 3], F32, tag="rstd3")
                    nc.scalar.activation(rstd3[:], r3[:], AF.Sqrt)
                    # lnp = (p3 - m*W1) * rstd + C
                    ln3 = dl.tile([128, 3], F32, tag="ln3")
                    nc.vector.tensor_tensor(out=ln3[:], in0=m3[:], in1=w1rep[:],
                                            op=AL.mult)
                    nc.vector.tensor_tensor(out=ln3[:], in0=p3[:], in1=ln3[:],
                                            op=AL.subtract)
                    nc.vector.tensor_tensor(out=ln3[:], in0=ln3[:],
                                            in1=rstd3[:], op=AL.mult)
                    nc.vector.tensor_tensor(out=ln3[:], in0=ln3[:], in1=c3rep[:],
                                            op=AL.add)
                    sig3 = dl.tile([128, 3], F32, tag="sig3")
                    nc.scalar.activation(sig3[:], ln3[:], AF.Sigmoid)
                    # logits = sig3 @ A / T
                    lg3 = dl.tile([128, 3], F32, tag="lg3")
                    nc.vector.tensor_tensor(
                        out=lg3[:], in0=sig3[:, 0:1].to_broadcast([128, 3]),
                        in1=arep[0][:], op=AL.mult)
                    tmp3 = dl.tile([128, 3], F32, tag="tmp3")
                    for i in (1, 2):
                        nc.vector.tensor_tensor(
                            out=tmp3[:],
                            in0=sig3[:, i:i + 1].to_broadcast([128, 3]),
                            in1=arep[i][:], op=AL.mult)
                        nc.vector.tensor_tensor(out=lg3[:], in0=lg3[:],
                                                in1=tmp3[:], op=AL.add)
                    # softmax over 3 + *3
                    mx1 = dl.tile([128, 1], F32, tag="mx1")
                    nc.vector.tensor_reduce(mx1[:], lg3[:],
                                            axis=mybir.AxisListType.X,
                                            op=AL.max)
                    nc.vector.tensor_scalar(out=lg3[:], in0=lg3[:],
                                            scalar1=mx1[:], scalar2=None,
                                            op0=AL.subtract)
                    e3 = dl.tile([128, 3], F32, tag="e3")
                    nc.scalar.activation(e3[:], lg3[:], AF.Exp)
                    se1 = dl.tile([128, 1], F32, tag="se1")
                    nc.vector.tensor_reduce(se1[:], e3[:],
                                            axis=mybir.AxisListType.X,
                                            op=AL.add)
                    rc1 = dl.tile([128, 1], F32, tag="rc1")
                    nc.vector.reciprocal(rc1[:], se1[:])
                    att3 = dl.tile([128, 3], F32, tag="att3")
                    nc.vector.tensor_scalar(out=att3[:], in0=e3[:],
                                            scalar1=rc1[:], scalar2=3.0,
                                            op0=AL.mult, op1=AL.mult)
                    # final combine
                    o_t = dl.tile([128, D], F32, tag="o_t")
                    nc.vector.tensor_scalar(out=o_t[:], in0=zall[:, 2, :],
                                            scalar1=att3[:, 2:3], scalar2=None,
                                            op0=AL.mult)
                    nc.vector.scalar_tensor_tensor(
                        out=o_t[:], in0=zall[:, 1, :], scalar=att3[:, 1:2],
                        in1=o_t[:], op0=AL.mult, op1=AL.add)
                    nc.vector.scalar_tensor_tensor(
                        out=o_t[:], in0=zall[:, 0, :], scalar=att3[:, 0:1],
                        in1=o_t[:], op0=AL.mult, op1=AL.add)
                    nc.sync.dma_start(out[t * 128:(t + 1) * 128, :], o_t[:])
    nc.compile()
    return nc


_CACHE = {}


def kernel(**inputs):
    x = np.asarray(inputs["x"], np.float32)
    graphs = {
        "low": (inputs["low_rows"], inputs["low_cols"], inputs["low_vals"]),
        "high": (inputs["high_rows"], inputs["high_cols"], inputs["high_vals"]),
    }
    per_core, schedules = _host_prep(x, graphs, None)

    wl = np.asarray(inputs["weight_low"], np.float32)
    wh = np.asarray(inputs["weight_high"], np.float32)
    wm = np.asarray(inputs["weight_mlp"], np.float32)
    att = {k: np.asarray(inputs[k], np.float32).reshape(D)
           for k in ("att_vec_low", "att_vec_high", "att_vec_mlp")}
    g_ = {k: np.asarray(inputs[k], np.float32) for k in
          ("ln_low_g", "ln_high_g", "ln_mlp_g")}
    b_ = {k: np.asarray(inputs[k], np.float32) for k in
          ("ln_low_b", "ln_high_b", "ln_mlp_b")}
    A = np.asarray(inputs["att_vec"], np.float32)

    # folded LN-projection weights: w_j = g_j * attvec_j ; W1 = sum(w),
    # C = b @ attvec
    wvec = np.stack([
        g_["ln_low_g"] * att["att_vec_low"],
        g_["ln_high_g"] * att["att_vec_high"],
        g_["ln_mlp_g"] * att["att_vec_mlp"],
    ])  # [3, 256]
    W1 = [float(w.sum()) for w in wvec]
    Cc = [float((b * a).sum()) for b, a in
          ((b_["ln_low_b"], att["att_vec_low"]),
           (b_["ln_high_b"], att["att_vec_high"]),
           (b_["ln_mlp_b"], att["att_vec_mlp"]))]
    wrep = np.tile(wvec.reshape(1, 3 * D), (128, 1)).astype(BF16)

    for c in range(NCORES):
        per_core[c]["wlow"] = wl.astype(BF16)
        per_core[c]["whigh"] = wh.astype(BF16)
        per_core[c]["wmlp"] = wm.astype(BF16)
        per_core[c]["wrep"] = wrep

    key = tuple(sorted((g, s["nslots"]) for g, s in schedules.items()))
    if key not in _CACHE:
        from concourse import bacc
        nc = bacc.Bacc(None, target_bir_lowering=False)
        _CACHE[key] = _build(nc, schedules, wl, wh, wm, wrep, W1, Cc,
                             A.tolist())
    nc = _CACHE[key]

    from concourse.bass_utils import run_bass_kernel_spmd
    res = run_bass_kernel_spmd(nc, per_core, core_ids=list(range(NCORES)))
    outp = np.concatenate([res.results[c]["out"] for c in range(NCORES)],
                          axis=0)
    return np.ascontiguousarray(outp[:N]).astype(np.float32)


if __name__ == "__main__":
    pass



# revision 16
# speedup vs baseline: 1.0761x; 1.0761x over previous
"""GNN message-passing (ACM module) Trainium2 kernel — 8 NeuronCores.

Strategy (per sharding hint): shard nodes (rows) across the 8 cores;
edges partitioned by destination row; weights/LN/attention replicated.

Per core, per graph:
  y = A @ X  computed as a chain of one-hot matmuls: edges sorted by
  (dest-tile, src-chunk); per 128-edge batch, lhsT = S^T [128 edges,
  128 dest-rows] built ON-CHIP from compact (dest, val) streams via
  tensor_scalar(iota, is_equal d, mult val) — 4x DVE mode; rhs = G
  [128 edges, 256] gathered rows of x (dma_gather, int16 chunk-local
  indices), accumulated in PSUM per dest tile.
Fused dense stage per dest tile (no DRAM roundtrip for y): y^T via PE
transposes, out_g = leaky(y_g @ W_g), out_mlp = leaky(x @ W_mlp) with
host-precomputed x^T, then a LayerNorm-projection attention epilogue
whose small-op chain is batched over EB tiles to avoid activation
table thrash.
"""
import os
import sys
import numpy as np

sys.path.insert(0, "/opt/trn_rl_repo")

import ml_dtypes  # noqa: E402

BF16 = ml_dtypes.bfloat16

# problem constants
N = 100000
D = 256
NCORES = 8
RPC = 12544            # rows per core (8*12544 = 100352 padded)
NPAD = RPC * NCORES
TPC = RPC // 128       # 98 dest tiles (of 128 rows) per core
CHUNK = 25088          # source chunk (int16-indexable)
NCH = 4
GRP = 2                # dest tiles per gather group
NGRP = TPC // GRP      # 49
EB = 7                 # epilogue batch (tiles per small-op chain)
EPS = 1e-5
T = 3.0


def _host_prep(x, graphs, weights):
    """Build all per-core device inputs. graphs = {name: (rows, cols, vals)}."""
    xpad = np.zeros((NPAD, D), np.float32)
    xpad[:N] = x
    xtab = xpad.astype(BF16)                      # gather table, replicated
    xT = np.ascontiguousarray(xpad.T.astype(BF16))  # [256, NPAD]

    per_core = [dict() for _ in range(NCORES)]
    for c in range(NCORES):
        per_core[c]["xtab"] = xtab
        per_core[c]["xT"] = np.ascontiguousarray(
            xT[:, c * RPC:(c + 1) * RPC])

    schedules = {}
    for gname, (rows, cols, vals) in graphs.items():
        rows = np.asarray(rows).astype(np.int64)
        cols = np.asarray(cols).astype(np.int64)
        vals = np.asarray(vals).astype(np.float32)
        core_of = rows // RPC
        # per-core edge lists sorted by (tile, chunk, col)
        edata = []
        counts = np.zeros((NCORES, TPC, NCH), np.int64)
        for c in range(NCORES):
            m = core_of == c
            r = rows[m] - c * RPC
            co = cols[m]
            v = vals[m]
            t = r >> 7
            ch = co // CHUNK
            order = np.lexsort((co, ch, t))
            r, co, v, t, ch = r[order], co[order], v[order], t[order], ch[order]
            np.add.at(counts[c], (t, ch), 1)
            edata.append((r, co, v, t, ch))
        # shared schedule: batches per (tile, chunk)
        mx = counts.max(axis=0)                       # [TPC, NCH]
        B = -(-mx // 128)                             # ceil
        B = np.maximum(B, 1)
        # slot order: group-major, then chunk, then tile-within-group
        S_tc = B * 128
        nslots = int(S_tc.sum())
        nbatch = int(B.sum())
        # slot base for (t, ch) in stream order
        base = np.zeros((TPC, NCH), np.int64)
        off = 0
        for g in range(NGRP):
            for ch in range(NCH):
                for tt in range(GRP):
                    t = g * GRP + tt
                    base[t, ch] = off
                    off += S_tc[t, ch]
        assert off == nslots

        idx_streams, d_streams, v_streams = [], [], []
        for c in range(NCORES):
            r, co, v, t, ch = edata[c]
            cnt = counts[c]
            # slot index per edge: base[t,ch] + rank within (t,ch)
            # edges are sorted by (t, ch), so rank = arange - start of group
            starts = np.zeros((TPC, NCH), np.int64)
            flat = cnt.reshape(-1)
            starts.reshape(-1)[:] = np.concatenate(([0], np.cumsum(flat)[:-1]))
            rank = np.arange(len(r)) - starts[t, ch]
            slot = base[t, ch] + rank
            # gather indices (chunk-local), pad slots -> 0
            idx = np.zeros(nslots, np.int16)
            idx[slot] = (co - ch * CHUNK).astype(np.int16)
            iw = np.zeros((16, nslots // 16), np.int16)
            sl = np.arange(nslots)
            iw[sl % 16, sl // 16] = idx
            idx_streams.append(np.tile(iw, (8, 1)))
            # compact S^T encoding: per slot, dest-within-tile and edge
            # value (f32: is_equal scalar must be f32). Pad slots get val 0
            # so the built one-hot col is all-zero.
            ds = np.zeros((128, nslots // 128), np.float32)
            vs = np.zeros((128, nslots // 128), np.float32)
            ds[slot & 127, slot >> 7] = (r & 127).astype(np.float32)
            vs[slot & 127, slot >> 7] = v
            d_streams.append(ds)
            v_streams.append(vs)
        schedules[gname] = dict(B=B, nslots=nslots, nbatch=nbatch, base=base)
        for c in range(NCORES):
            per_core[c][f"idx_{gname}"] = idx_streams[c]
            per_core[c][f"dst_{gname}"] = d_streams[c]
            per_core[c][f"val_{gname}"] = v_streams[c]
    iota = np.tile(np.arange(128, dtype=np.float32)[None, :], (128, 1))
    for c in range(NCORES):
        per_core[c]["iota"] = iota.astype(BF16)
        per_core[c]["ident"] = np.eye(128, dtype=np.float32).astype(BF16)
    return per_core, schedules


def _build(nc_mod, schedules, wl, wh, wm, wrep, W1, Cc, A, reps=1):
    """Build the Bass graph. wrep [128, 3*D] bf16 LN-proj weights. W1, Cc:
    len-3 float lists. A: [3,3] floats. reps>1 repeats the body in-NEFF
    for dispatch-overhead-free timing."""
    import concourse.bass as bass
    import concourse.mybir as mybir
    import concourse.tile as tile

    nc = nc_mod
    F32 = mybir.dt.float32
    BF = mybir.dt.bfloat16
    AL = mybir.AluOpType
    AF = mybir.ActivationFunctionType

    xtab = nc.dram_tensor("xtab", [NPAD, D], BF, kind="ExternalInput")
    xT = nc.dram_tensor("xT", [D, RPC], BF, kind="ExternalInput")
    w_in = {}
    for nm in ("wlow", "whigh", "wmlp"):
        w_in[nm] = nc.dram_tensor(nm, [D, D], BF, kind="ExternalInput")
    wrep_in = nc.dram_tensor("wrep", [128, 3 * D], BF, kind="ExternalInput")
    iota_in = nc.dram_tensor("iota", [128, 128], BF, kind="ExternalInput")
    ident_in = nc.dram_tensor("ident", [128, 128], BF, kind="ExternalInput")
    gins = {}
    for g in ("low", "high"):
        sch = schedules[g]
        gins[g] = dict(
            idx=nc.dram_tensor(f"idx_{g}", [128, sch["nslots"] // 16],
                               mybir.dt.int16, kind="ExternalInput"),
            dst=nc.dram_tensor(f"dst_{g}", [128, sch["nslots"] // 128], F32,
                               kind="ExternalInput"),
            val=nc.dram_tensor(f"val_{g}", [128, sch["nslots"] // 128], F32,
                               kind="ExternalInput"),
        )
    out = nc.dram_tensor("out", [RPC, D], F32, kind="ExternalOutput")

    with tile.TileContext(nc) as tc:
      for _rep in range(reps):
        with (
            tc.tile_pool(name="segc", bufs=1) as segc,
            tc.tile_pool(name="seg", bufs=2) as seg,
            tc.tile_pool(name="sps", bufs=1, space="PSUM") as sps,
            tc.tile_pool(name="dl", bufs=3) as dl,
            tc.tile_pool(name="dps", bufs=1, space="PSUM") as dps,
            tc.tile_pool(name="tps", bufs=1, space="PSUM") as tps,
            tc.tile_pool(name="eb", bufs=2) as eb,
        ):
            # ---- constants ----
            iota_t = segc.tile([128, 128], BF, tag="iota")
            nc.sync.dma_start(iota_t[:], iota_in[:])
            ident_t = segc.tile([128, 128], BF, tag="ident")
            nc.sync.dma_start(ident_t[:], ident_in[:])
            w_t = {}
            for nm in ("wlow", "whigh", "wmlp"):
                w_t[nm] = segc.tile([128, 2, D], BF, name=f"w_{nm}", tag=nm)
                for k in range(2):
                    nc.sync.dma_start(w_t[nm][:, k, :],
                                      w_in[nm][k * 128:(k + 1) * 128, :])
            wrep_t = segc.tile([128, 3 * D], BF, tag="wrep")
            nc.sync.dma_start(wrep_t[:], wrep_in[:])
            # batched epilogue constants [128, EB, 3]
            w1rep = segc.tile([128, EB, 3], F32, tag="w1rep")
            c3rep = segc.tile([128, EB, 3], F32, tag="c3rep")
            arep = [segc.tile([128, EB, 3], F32, name=f"arep{j}", tag=f"arep{j}")
                    for j in range(3)]
            for e in range(EB):
                for j in range(3):
                    nc.gpsimd.memset(w1rep[:, e, j:j + 1], float(W1[j]))
                    nc.gpsimd.memset(c3rep[:, e, j:j + 1], float(Cc[j]))
                    for i in range(3):
                        nc.gpsimd.memset(arep[i][:, e, j:j + 1],
                                         float(A[i][j] / T))

            ebuf = {}
            for grp in range(NGRP):
                tiles = [grp * GRP + tt for tt in range(GRP)]
                # ---- segment-sum stage, both graphs ----
                ps = {}
                for g in ("low", "high"):
                    sch = schedules[g]
                    B = sch["B"]
                    base = sch["base"]
                    s0 = int(base[tiles[0], 0])
                    s1 = int(base[tiles[-1], NCH - 1] +
                             B[tiles[-1], NCH - 1] * 128)
                    nsl = s1 - s0
                    nb = nsl // 128
                    g_t = seg.tile([128, nsl // 128, D], BF, tag="G")
                    idx_t = seg.tile([128, nsl // 16], mybir.dt.int16,
                                     tag="idx")
                    d_t = seg.tile([128, nb], F32, tag="dst")
                    v_t = seg.tile([128, nb], F32, tag="val")
                    st_t = seg.tile([128, nsl], BF, tag="st")
                    nc.sync.dma_start(idx_t[:],
                                      gins[g]["idx"][:, s0 // 16:s1 // 16])
                    nc.sync.dma_start(d_t[:],
                                      gins[g]["dst"][:, s0 // 128:s1 // 128])
                    nc.sync.dma_start(v_t[:],
                                      gins[g]["val"][:, s0 // 128:s1 // 128])
                    for ch in range(NCH):
                        c0 = int(base[tiles[0], ch])
                        c1 = int(base[tiles[-1], ch] + B[tiles[-1], ch] * 128)
                        nid = c1 - c0
                        nc.gpsimd.dma_gather(
                            out_ap=g_t[:, (c0 - s0) // 128:(c1 - s0) // 128, :],
                            in_ap=xtab[ch * CHUNK:(ch + 1) * CHUNK, :],
                            idxs_ap=idx_t[:, (c0 - s0) // 16:(c1 - s0) // 16],
                            num_idxs=nid, num_idxs_reg=nid,
                            elem_size=D, single_packet=False,
                        )
                    for t in tiles:
                        ps[(g, t)] = sps.tile(
                            [128, D], F32, name=f"ps_{g}_{t}",
                            tag=f"ps_{g}_{t % GRP}")
                    for ch in range(NCH):
                        for t in tiles:
                            b0 = int(base[t, ch])
                            for b in range(int(B[t, ch])):
                                sb = b0 + b * 128 - s0
                                # build S^T window on-chip (4x DVE mode)
                                nc.vector.tensor_scalar(
                                    out=st_t[:, sb:sb + 128],
                                    in0=iota_t[:],
                                    scalar1=d_t[:, sb // 128:sb // 128 + 1],
                                    scalar2=v_t[:, sb // 128:sb // 128 + 1],
                                    op0=AL.is_equal, op1=AL.mult)
                                nc.tensor.matmul(
                                    ps[(g, t)][:],
                                    st_t[:, sb:sb + 128],
                                    g_t[:, sb // 128, :],
                                    start=(ch == 0 and b == 0),
                                    stop=(ch == NCH - 1 and
                                          b == int(B[t, ch]) - 1),
                                )

                # ---- fused dense + epilogue accumulation per tile ----
                for t in tiles:
                    e = t % EB
                    if e == 0:
                        ebuf = dict(
                            zall=eb.tile([128, EB, 3, D], BF, name="zallg",
                                         tag="zall"),
                            m3=eb.tile([128, EB, 3], F32, name="m3g",
                                       tag="m3"),
                            ss3=eb.tile([128, EB, 3], F32, name="ss3g",
                                        tag="ss3"),
                            p3=eb.tile([128, EB, 3], F32, name="p3g",
                                       tag="p3"),
                        )
                    ps3 = {}
                    for bi, (gname, wname) in enumerate(
                            (("low", "wlow"), ("high", "whigh"))):
                        yg = dl.tile([128, D], BF, tag=f"y{bi}")
                        nc.scalar.copy(yg[:], ps[(gname, t)][:])
                        for k in range(2):
                            tr = tps.tile([128, 128], BF, tag="tr")
                            nc.tensor.transpose(
                                tr[:], yg[:, k * 128:(k + 1) * 128],
                                ident_t[:])
                            yT = dl.tile([128, 128], BF, tag=f"yT{bi}{k}")
                            nc.scalar.copy(yT[:], tr[:])
                            if k == 0:
                                ps3[bi] = dps.tile([128, D], F32,
                                                   name=f"eps{bi}t",
                                                   tag=f"eps{bi}")
                            nc.tensor.matmul(
                                ps3[bi][:], yT[:], w_t[wname][:, k, :],
                                start=(k == 0), stop=(k == 1))
                    ps3[2] = dps.tile([128, D], F32, name="eps2t", tag="eps2")
                    for k in range(2):
                        xTt = dl.tile([128, 128], BF, tag=f"xTt{k}")
                        nc.sync.dma_start(
                            xTt[:],
                            xT[k * 128:(k + 1) * 128, t * 128:(t + 1) * 128])
                        nc.tensor.matmul(
                            ps3[2][:], xTt[:], w_t["wmlp"][:, k, :],
                            start=(k == 0), stop=(k == 1))

                    sr3 = dl.tile([128, 3], F32, tag="sr3")
                    sz3 = dl.tile([128, 3], F32, tag="sz3")
                    rsc = dl.tile([128, D], BF, tag="rsc")
                    z01 = dl.tile([128, D], BF, tag="z01")
                    for bi in range(3):
                        nc.scalar.activation(rsc[:], ps3[bi][:], AF.Relu,
                                             accum_out=sr3[:, bi:bi + 1])
                        nc.scalar.activation(z01[:], ps3[bi][:], AF.Copy,
                                             scale=0.01,
                                             accum_out=sz3[:, bi:bi + 1])
                        nc.vector.tensor_tensor(
                            out=ebuf["zall"][:, e, bi, :], in0=z01[:],
                            in1=ps3[bi][:], op=AL.max)
                    # mean = (0.99*sum_relu + sum_0.01x) / D
                    m3 = ebuf["m3"][:, e]
                    nc.vector.tensor_scalar(out=m3, in0=sr3[:],
                                            scalar1=0.99 / D, scalar2=None,
                                            op0=AL.mult)
                    nc.vector.scalar_tensor_tensor(
                        out=m3, in0=sz3[:], scalar=1.0 / D, in1=m3,
                        op0=AL.mult, op1=AL.add)
                    # sq sums and proj sums
                    sq = dl.tile([128, 3, D], BF, tag="sq")
                    nc.vector.tensor_tensor(out=sq[:],
                                            in0=ebuf["zall"][:, e],
                                            in1=ebuf["zall"][:, e],
                                            op=AL.mult)
                    nc.vector.tensor_reduce(ebuf["ss3"][:, e], sq[:],
                                            axis=mybir.AxisListType.X,
                                            op=AL.add)
                    pj = dl.tile([128, 3, D], BF, tag="pj")
                    nc.vector.tensor_tensor(
                        out=pj[:], in0=ebuf["zall"][:, e],
                        in1=wrep_t[:].rearrange("p (a d) -> p a d", a=3),
                        op=AL.mult)
                    nc.vector.tensor_reduce(ebuf["p3"][:, e], pj[:],
                                            axis=mybir.AxisListType.X,
                                            op=AL.add)

                    # ---- batched small-op chain every EB tiles ----
                    if e == EB - 1:
                        t0 = t - EB + 1
                        m3g = ebuf["m3"][:]          # [128, EB, 3]
                        ss3g = ebuf["ss3"][:]
                        p3g = ebuf["p3"][:]
                        v3 = dl.tile([128, EB, 3], F32, tag="v3")
                        nc.vector.tensor_tensor(out=v3[:], in0=m3g, in1=m3g,
                                                op=AL.mult)
                        nc.vector.scalar_tensor_tensor(
                            out=v3[:], in0=ss3g, scalar=1.0 / D, in1=v3[:],
                            op0=AL.mult, op1=AL.subtract)
                        nc.vector.tensor_scalar(out=v3[:], in0=v3[:],
                                                scalar1=EPS, scalar2=None,
                                                op0=AL.add)
                        r3 = dl.tile([128, EB, 3], F32, tag="r3")
                        nc.vector.reciprocal(r3[:], v3[:])
                        rstd = dl.tile([128, EB, 3], F32, tag="rstd")
                        nc.scalar.activation(rstd[:], r3[:], AF.Sqrt)
                        # lnp = (p3 - m*W1) * rstd + C
                        ln3 = dl.tile([128, EB, 3], F32, tag="ln3")
                        nc.vector.tensor_tensor(out=ln3[:], in0=m3g,
                                                in1=w1rep[:], op=AL.mult)
                        nc.vector.tensor_tensor(out=ln3[:], in0=p3g,
                                                in1=ln3[:], op=AL.subtract)
                        nc.vector.tensor_tensor(out=ln3[:], in0=ln3[:],
                                                in1=rstd[:], op=AL.mult)
                        nc.vector.tensor_tensor(out=ln3[:], in0=ln3[:],
                                                in1=c3rep[:], op=AL.add)
                        sig3 = dl.tile([128, EB, 3], F32, tag="sig3")
                        nc.scalar.activation(sig3[:], ln3[:], AF.Sigmoid)
                        # logits = sig3 @ A / T
                        lg3 = dl.tile([128, EB, 3], F32, tag="lg3")
                        nc.vector.tensor_tensor(
                            out=lg3[:],
                            in0=sig3[:, :, 0:1].to_broadcast([128, EB, 3]),
                            in1=arep[0][:], op=AL.mult)
                        tmp3 = dl.tile([128, EB, 3], F32, tag="tmp3")
                        for i in (1, 2):
                            nc.vector.tensor_tensor(
                                out=tmp3[:],
                                in0=sig3[:, :, i:i + 1].to_broadcast(
                                    [128, EB, 3]),
                                in1=arep[i][:], op=AL.mult)
                            nc.vector.tensor_tensor(out=lg3[:], in0=lg3[:],
                                                    in1=tmp3[:], op=AL.add)
                        # softmax over 3 (innermost) + *3
                        mx1 = dl.tile([128, EB, 1], F32, tag="mx1")
                        nc.vector.tensor_reduce(mx1[:], lg3[:],
                                                axis=mybir.AxisListType.X,
                                                op=AL.max)
                        nc.vector.tensor_tensor(
                            out=lg3[:], in0=lg3[:],
                            in1=mx1[:].to_broadcast([128, EB, 3]),
                            op=AL.subtract)
                        e3 = dl.tile([128, EB, 3], F32, tag="e3")
                        nc.scalar.activation(e3[:], lg3[:], AF.Exp)
                        se1 = dl.tile([128, EB, 1], F32, tag="se1")
                        nc.vector.tensor_reduce(se1[:], e3[:],
                                                axis=mybir.AxisListType.X,
                                                op=AL.add)
                        rc1 = dl.tile([128, EB, 1], F32, tag="rc1")
                        nc.vector.reciprocal(rc1[:], se1[:])
                        att3 = dl.tile([128, EB, 3], F32, tag="att3")
                        nc.vector.scalar_tensor_tensor(
                            out=att3[:], in0=e3[:], scalar=3.0,
                            in1=rc1[:].to_broadcast([128, EB, 3]),
                            op0=AL.mult, op1=AL.mult)
                        # final combine + store per tile in the batch
                        for ee in range(EB):
                            tt_ = t0 + ee
                            o_t = dl.tile([128, D], F32, tag="o_t")
                            nc.vector.tensor_scalar(
                                out=o_t[:], in0=ebuf["zall"][:, ee, 2, :],
                                scalar1=att3[:, ee, 2:3], scalar2=None,
                                op0=AL.mult)
                            nc.vector.scalar_tensor_tensor(
                                out=o_t[:], in0=ebuf["zall"][:, ee, 1, :],
                                scalar=att3[:, ee, 1:2], in1=o_t[:],
                                op0=AL.mult, op1=AL.add)
                            nc.vector.scalar_tensor_tensor(
                                out=o_t[:], in0=ebuf["zall"][:, ee, 0, :],
                                scalar=att3[:, ee, 0:1], in1=o_t[:],
                                op0=AL.mult, op1=AL.add)
                            nc.sync.dma_start(
                                out[tt_ * 128:(tt_ + 1) * 128, :], o_t[:])
    nc.compile()
    return nc


_CACHE = {}
_LAST = {}
_last_per_core = None


def build_reps(reps):
    """Build (or fetch) an nc whose body repeats `reps` times, for timing.
    Must be called after kernel() has populated _LAST."""
    schedules = _LAST["schedules"]
    key = tuple(sorted((g, s["nslots"]) for g, s in schedules.items())) \
        + (reps,)
    if key not in _CACHE:
        from concourse import bacc
        nc = bacc.Bacc(None, target_bir_lowering=False)
        _CACHE[key] = _build(nc, schedules, *_LAST["wargs"], reps=reps)
    return _CACHE[key]


def kernel(**inputs):
    x = np.asarray(inputs["x"], np.float32)
    graphs = {
        "low": (inputs["low_rows"], inputs["low_cols"], inputs["low_vals"]),
        "high": (inputs["high_rows"], inputs["high_cols"], inputs["high_vals"]),
    }
    per_core, schedules = _host_prep(x, graphs, None)

    wl = np.asarray(inputs["weight_low"], np.float32)
    wh = np.asarray(inputs["weight_high"], np.float32)
    wm = np.asarray(inputs["weight_mlp"], np.float32)
    att = {k: np.asarray(inputs[k], np.float32).reshape(D)
           for k in ("att_vec_low", "att_vec_high", "att_vec_mlp")}
    g_ = {k: np.asarray(inputs[k], np.float32) for k in
          ("ln_low_g", "ln_high_g", "ln_mlp_g")}
    b_ = {k: np.asarray(inputs[k], np.float32) for k in
          ("ln_low_b", "ln_high_b", "ln_mlp_b")}
    A = np.asarray(inputs["att_vec"], np.float32)

    # folded LN-projection weights: w_j = g_j * attvec_j ; W1 = sum(w),
    # C = b @ attvec
    wvec = np.stack([
        g_["ln_low_g"] * att["att_vec_low"],
        g_["ln_high_g"] * att["att_vec_high"],
        g_["ln_mlp_g"] * att["att_vec_mlp"],
    ])  # [3, 256]
    W1 = [float(w.sum()) for w in wvec]
    Cc = [float((b * a).sum()) for b, a in
          ((b_["ln_low_b"], att["att_vec_low"]),
           (b_["ln_high_b"], att["att_vec_high"]),
           (b_["ln_mlp_b"], att["att_vec_mlp"]))]
    wrep = np.tile(wvec.reshape(1, 3 * D), (128, 1)).astype(BF16)

    for c in range(NCORES):
        per_core[c]["wlow"] = wl.astype(BF16)
        per_core[c]["whigh"] = wh.astype(BF16)
        per_core[c]["wmlp"] = wm.astype(BF16)
        per_core[c]["wrep"] = wrep

    global _last_per_core
    _last_per_core = per_core
    _LAST["schedules"] = schedules
    _LAST["wargs"] = (wl, wh, wm, wrep, W1, Cc, A.tolist())

    key = tuple(sorted((g, s["nslots"]) for g, s in schedules.items())) + (1,)
    if key not in _CACHE:
        from concourse import bacc
        nc = bacc.Bacc(None, target_bir_lowering=False)
        _CACHE[key] = _build(nc, schedules, wl, wh, wm, wrep, W1, Cc,
                             A.tolist(), reps=1)
    nc = _CACHE[key]

    from concourse.bass_utils import run_bass_kernel_spmd
    res = run_bass_kernel_spmd(nc, per_core, core_ids=list(range(NCORES)))
    outp = np.concatenate([res.results[c]["out"] for c in range(NCORES)],
                          axis=0)
    return np.ascontiguousarray(outp[:N]).astype(np.float32)


if __name__ == "__main__":
    pass
